# revision 2
# baseline (speedup 1.0000x reference)
"""Multi-head causal self-attention on 8 Trainium2 NeuronCores.

Problem: B=4, T=2048, D=1024, H=16 heads, Hd=64. fp32.
Sharding: core c handles batch b = c//2 and head-group g = c%2 (8 heads,
512 channels). Each core computes a partial output (its head-group's
contribution to x @ Wo); the host sums head-group pairs and adds bo.

Per-core algorithm (all layouts chosen so no on-chip transposes are
needed after the single x -> x^T transpose):
  x^T  [D=1024, T]   via XBAR DMA transpose, split across the sync and
                     scalar HWDGE rings so weights + transposes overlap
  Q^T  [C=512, T]    = matmul(lhsT=Wq chunk, rhs=x^T)   (head h at partitions
  K^T  [C=512, T]      64*(h%2) .. of chunk h//2)
  V'   [T, 8*65]     = matmul(lhsT=x^T chunk, rhs=Wv), per head [V(64) | 1]
  S^T  [k,q]         = matmul(lhsT=K^T block, rhs=Q^T span); the two heads
                       of a pair are row-tiled (partitions 0-63 / 64-127)
                       and run concurrently in the PE array
  E = exp((S^T+mask)/8)  on ScalarE, PSUM->SBUF
  ctx' [65, q]       = matmul(lhsT=V' block, rhs=E)  accumulated over k blocks
                       row 64 = softmax denominator (ones-column trick)
  ctx^T normalized via reciprocal + DRAM-bounce partition broadcast
  out  [T, D]        = matmul(lhsT=ctx^T chunk, rhs=Wo chunk), DMA out

Phase C (attention) is ScalarE-exp-throughput-bound, so the emission
order software-pipelines everything else into its PE idle time:
  - per head pair hp: emit K(hp,s), Q(hp,s) groups just before C(hp,s)
    (spans ascending) so the first exp can start ~25us into the kernel
    and later projections fill PE gaps during earlier C spans.
  - hp==3 runs spans DESCENDING and the output projection for span s is
    emitted right after norm(3,s), so out-proj overlaps the remaining
    attention work instead of serializing at the end.
  - One shared [128,512] PSUM pool (bufs=2) serves V'/QK/out-proj so
    those matmuls can coexist with attention PSUM (st 4 + csA/csB 2 + 2
    = 8 banks).
Causality: only k-blocks with k0 <= q_span_end are computed; the <=4
diagonal blocks per span get a multiplicative staircase mask applied to
the first 128 columns only (the rest of the window is always valid).
"""

import sys

for _p in ("/opt/trn_rl_repo", "/root/.axon_site/_ro/trn_rl_repo"):
    if _p not in sys.path:
        sys.path.append(_p)

import numpy as np

import concourse.bacc as bacc
import concourse.mybir as mybir
import concourse.tile as tile
from concourse.bass_utils import run_bass_kernel_spmd

FP32 = mybir.dt.float32
BF16 = mybir.dt.bfloat16
P = 128
T = 2048  # sequence length
D = 1024  # model dim
C = 512   # channels per core (8 heads)
H = 8     # heads per core
HD = 64   # head dim
N_CORES = 8
NSPAN = 4          # q spans of 512
SPAN = 512
NKB = 16           # k blocks of 128

_program = None


def _build():
    nc = bacc.Bacc()
    x_d = nc.declare_dram_parameter("x", [T, D], BF16, isOutput=False)
    wq_d = nc.declare_dram_parameter("wq", [D, C], BF16, isOutput=False)
    wk_d = nc.declare_dram_parameter("wk", [D, C], BF16, isOutput=False)
    wv_d = nc.declare_dram_parameter("wv", [D, C], BF16, isOutput=False)
    wo_d = nc.declare_dram_parameter("wo", [C, D], BF16, isOutput=False)
    mask_d = nc.declare_dram_parameter("mask", [P, 1024], BF16, isOutput=False)
    out_d = nc.declare_dram_parameter("out", [T, D], FP32, isOutput=True)

    Exp = mybir.ActivationFunctionType.Exp

    from contextlib import ExitStack

    with tile.TileContext(nc) as tc, ExitStack() as persist:
        const_pool = persist.enter_context(tc.tile_pool(name="const", bufs=1))
        qkt_pool = persist.enter_context(tc.tile_pool(name="qkt", bufs=1))
        vp_pool = persist.enter_context(tc.tile_pool(name="vp", bufs=1))
        persist_w = persist.enter_context(tc.tile_pool(name="pw", bufs=1))
        ctxT_pool = persist.enter_context(tc.tile_pool(name="ctxT", bufs=1))
        xt_pool = persist.enter_context(tc.tile_pool(name="xt", bufs=1))

        # ---- persistent SBUF tiles --------------------------------------
        mask_sb = const_pool.tile([P, 1024], BF16, tag="mask")
        wv_sb = persist_w.tile([P, 8, C], BF16, tag="wv")
        wq_sb = persist_w.tile([P, 8, C], BF16, tag="wq")
        wk_sb = persist_w.tile([P, 8, C], BF16, tag="wk")
        wo_sb = persist_w.tile([P, 4, D], BF16, tag="wo")
        qt = [qkt_pool.tile([P, T], BF16, tag=f"qt{i}", name=f"qt{i}") for i in range(4)]
        kt = [qkt_pool.tile([P, T], BF16, tag=f"kt{i}", name=f"kt{i}") for i in range(4)]
        vp = [vp_pool.tile([P, H * 65], BF16, tag=f"vp{t}", name=f"vp{t}") for t in range(NKB)]
        ctxT = [ctxT_pool.tile([P, T], BF16, tag=f"ct{i}", name=f"ct{i}")
                for i in range(4)]
        xt = [xt_pool.tile([P, T], BF16, tag=f"xt{j}", name=f"xt{j}") for j in range(8)]

        # ---- startup DMAs: weights first (small), transposes split over
        # the two HWDGE rings (sync + scalar) so nothing serializes behind
        # the 16 x-transposes.
        nc.sync.dma_start(wv_sb[:], wv_d.rearrange("(o p) c -> p o c", p=P))
        nc.sync.dma_start(wq_sb[:], wq_d.rearrange("(o p) c -> p o c", p=P))
        nc.sync.dma_start(wk_sb[:], wk_d.rearrange("(o p) c -> p o c", p=P))
        nc.sync.dma_start(mask_sb[:], mask_d[:])
        for th in range(2):
            for j in range(8):
                tsl = slice(th * (T // 2), (th + 1) * (T // 2))
                eng = nc.sync if j % 2 == 0 else nc.scalar
                eng.dma_start_transpose(
                    xt[j][:, tsl], x_d[tsl, j * P:(j + 1) * P])
        nc.sync.dma_start(wo_sb[:], wo_d.rearrange("(o p) d -> p o d", p=P))

        # ones columns of V' (value 1.0 at element 64 of each head block)
        for t in range(NKB):
            nc.gpsimd.memset(vp[t][:], 1.0)

        with (
            tc.tile_pool(name="proj", bufs=2, space="PSUM") as proj_pool,
            tc.tile_pool(name="stps", bufs=2, space="PSUM") as st_pool,
            tc.tile_pool(name="csA", bufs=1, space="PSUM") as csA_pool,
            tc.tile_pool(name="csB", bufs=1, space="PSUM") as csB_pool,
            tc.tile_pool(name="epool", bufs=6) as e_pool,
            tc.tile_pool(name="npool", bufs=2) as n_pool,
            tc.tile_pool(name="rdram", bufs=2, space="DRAM") as rdram_pool,
            tc.tile_pool(name="opool", bufs=2) as o_pool,
        ):
            def emit_vprime(t):
                # V' for token block t: [128t, 8*65] with ones col at 64
                ps = proj_pool.tile([P, C], FP32, tag="proj")
                for j in range(8):
                    nc.tensor.matmul(
                        ps[:],
                        xt[j][:, t * P:(t + 1) * P],
                        wv_sb[:, j, :],
                        start=(j == 0), stop=(j == 7),
                    )
                dst = vp[t].rearrange("p (h e) -> p h e", e=65)[:, :, 0:64]
                nc.vector.tensor_copy(dst, ps.rearrange("p (h e) -> p h e", e=64))

            def emit_qk_group(dst, wsb, hp, s):
                ps = proj_pool.tile([P, SPAN], FP32, tag="proj")
                for j in range(8):
                    nc.tensor.matmul(
                        ps[:],
                        wsb[:, j, hp * P:(hp + 1) * P],
                        xt[j][:, s * SPAN:(s + 1) * SPAN],
                        start=(j == 0), stop=(j == 7),
                    )
                nc.vector.tensor_copy(dst[hp][:, s * SPAN:(s + 1) * SPAN], ps[:])

            def emit_attn_span(hp, s):
                hA, hB = 2 * hp, 2 * hp + 1
                csA = csA_pool.tile([P, SPAN], FP32, tag="csA")
                csB = csB_pool.tile([P, SPAN], FP32, tag="csB")
                nkb = 4 * s + 4
                for kb in range(nkb):
                    ksl = slice(kb * P, (kb + 1) * P)
                    d = max(0, kb - 4 * s)      # diagonal offset 0..3
                    q0 = s * SPAN + 128 * d     # valid q start
                    w = SPAN - 128 * d          # valid width
                    qsl = slice(q0, (s + 1) * SPAN)
                    st = st_pool.tile([P, 1024], FP32, tag="st")
                    st3 = st.rearrange("p (b q) -> p b q", b=2)[:, :, 0:w]
                    # the two heads run concurrently (row-tiled at
                    # partitions 0-63 / 64-127)
                    nc.tensor.matmul(st[:, 0:w], kt[hp][0:64, ksl],
                                     qt[hp][0:64, qsl],
                                     start=True, stop=True)
                    nc.tensor.matmul(st[:, 512:512 + w], kt[hp][64:128, ksl],
                                     qt[hp][64:128, qsl],
                                     start=True, stop=True)
                    e = e_pool.tile([P, 1024], BF16, tag="e")
                    e3 = e.rearrange("p (b q) -> p b q", b=2)[:, :, 0:w]
                    nc.scalar.activation(e3, st3, Exp, scale=0.125)
                    if d > 0 or kb == 4 * s:
                        # staircase only affects the first 128 columns of
                        # the valid window (beyond that q-k >= 128 always)
                        e3m = e.rearrange("p (b q) -> p b q", b=2)[:, :, 0:128]
                        m3 = mask_sb[:, None, 384:512]
                        nc.vector.tensor_mul(
                            e3m, e3m, m3.to_broadcast((P, 2, 128)))
                    co = 128 * d
                    nc.tensor.matmul(csA[0:65, co:SPAN],
                                     vp[kb][:, hA * 65:(hA + 1) * 65],
                                     e[:, 0:w],
                                     start=(kb == 0), stop=(kb == nkb - 1))
                    nc.tensor.matmul(csB[0:65, co:SPAN],
                                     vp[kb][:, hB * 65:(hB + 1) * 65],
                                     e[:, 512:512 + w],
                                     start=(kb == 0), stop=(kb == nkb - 1))
                # normalize: rows 0..63 / row 64 (ones-column rowsum).
                # reciprocal_approx_fast is broken at nonzero base
                # partition: broadcast first (DRAM bounce), recip at 0.
                qsl = slice(s * SPAN, (s + 1) * SPAN)
                rs = n_pool.tile([P, 1024], FP32, tag="rs")
                rsA = n_pool.tile([P, SPAN], FP32, tag="rsA")
                rsB = n_pool.tile([P, SPAN], FP32, tag="rsB")
                rrA = n_pool.tile([P, SPAN], FP32, tag="rrA")
                rrB = n_pool.tile([P, SPAN], FP32, tag="rrB")
                tmpB = n_pool.tile([P, SPAN], BF16, tag="tmpB")
                nc.vector.tensor_copy(rs[64:65, 0:512], csA[64:65, :])
                nc.vector.tensor_copy(rs[64:65, 512:1024], csB[64:65, :])
                rd = rdram_pool.tile([1024], FP32, tag="rd")
                nc.sync.dma_start(rd[None, :], rs[64:65, :])
                nc.sync.dma_start(
                    rsA[0:64, :], rd[None, 0:512].to_broadcast((64, 512)))
                nc.sync.dma_start(
                    rsB[0:64, :], rd[None, 512:1024].to_broadcast((64, 512)))
                nc.vector.reciprocal_approx_fast(rrA[0:64, :], rsA[0:64, :])
                nc.vector.reciprocal_approx_fast(rrB[0:64, :], rsB[0:64, :])
                nc.vector.tensor_mul(ctxT[hp][0:64, qsl],
                                     csA[0:64, :], rrA[0:64, :])
                nc.vector.tensor_mul(tmpB[0:64, :],
                                     csB[0:64, :], rrB[0:64, :])
                nc.sync.dma_start(ctxT[hp][64:128, qsl], tmpB[0:64, :])

            def emit_out_span(s):
                # output projection for the 4 token blocks of span s
                for qb in range(4 * s, 4 * s + 4):
                    ot = o_pool.tile([P, 2, SPAN], FP32, tag="ot")
                    for nh in range(2):
                        ps = proj_pool.tile([P, SPAN], FP32, tag="proj")
                        for hp in range(4):
                            nc.tensor.matmul(
                                ps[:],
                                ctxT[hp][:, qb * P:(qb + 1) * P],
                                wo_sb[:, hp, nh * SPAN:(nh + 1) * SPAN],
                                start=(hp == 0), stop=(hp == 3),
                            )
                        nc.vector.tensor_copy(ot[:, nh, :], ps[:])
                    eng = nc.scalar if s == 0 and qb % 2 == 1 else nc.sync
                    eng.dma_start(
                        out_d[qb * P:(qb + 1) * P, :],
                        ot.rearrange("p a b -> p (a b)"))

            # ---- emission schedule ------------------------------------
            for hp in range(3):
                for s in range(NSPAN):
                    if hp == 0:
                        for t in range(4 * s, 4 * s + 4):
                            emit_vprime(t)
                    emit_qk_group(kt, wk_sb, hp, s)
                    emit_qk_group(qt, wq_sb, hp, s)
                    emit_attn_span(hp, s)
            for s in range(NSPAN):
                emit_qk_group(kt, wk_sb, 3, s)
                emit_qk_group(qt, wq_sb, 3, s)
            for s in reversed(range(NSPAN)):
                emit_attn_span(3, s)
                emit_out_span(s)

    nc.compile()
    return nc


def _get_program():
    global _program
    if _program is None:
        _program = _build()
    return _program


def _make_mask():
    import ml_dtypes
    j = np.arange(1024)[None, :]
    k = np.arange(P)[:, None]
    return np.where(j >= k + 384, 1.0, 0.0).astype(ml_dtypes.bfloat16)


def _make_in_maps(x, Wq, Wk, Wv, Wo):
    import ml_dtypes
    bf16 = ml_dtypes.bfloat16
    mask = _make_mask()
    in_maps = []
    for c in range(N_CORES):
        b, g = c // 2, c % 2
        cols = slice(g * C, (g + 1) * C)
        in_maps.append({
            "x": np.ascontiguousarray(np.asarray(x[b], np.float32).astype(bf16)),
            "wq": np.ascontiguousarray(np.asarray(Wq[:, cols], np.float32).astype(bf16)),
            "wk": np.ascontiguousarray(np.asarray(Wk[:, cols], np.float32).astype(bf16)),
            "wv": np.ascontiguousarray(np.asarray(Wv[:, cols], np.float32).astype(bf16)),
            "wo": np.ascontiguousarray(np.asarray(Wo[cols, :], np.float32).astype(bf16)),
            "mask": mask,
        })
    return in_maps


def _combine(results, bo, B):
    out = np.empty((B, T, D), dtype=np.float32)
    bo = np.asarray(bo, dtype=np.float32)
    for b in range(B):
        out[b] = results[2 * b]["out"] + results[2 * b + 1]["out"] + bo
    return out


def kernel(x, Wq, Wk, Wv, Wo, bo):
    x = np.asarray(x)
    nc = _get_program()
    in_maps = _make_in_maps(x, Wq, Wk, Wv, Wo)
    res = run_bass_kernel_spmd(nc, in_maps, core_ids=list(range(N_CORES)))
    return _combine(res.results, bo, x.shape[0])


def kernel_traced(x, Wq, Wk, Wv, Wo, bo):
    """Like kernel() but also returns the BassKernelResults (with
    exec_time_ns when NTFF tracing is available)."""
    x = np.asarray(x)
    nc = _get_program()
    in_maps = _make_in_maps(x, Wq, Wk, Wv, Wo)
    res = run_bass_kernel_spmd(nc, in_maps, core_ids=list(range(N_CORES)),
                               trace=True)
    return _combine(res.results, bo, x.shape[0]), res


# revision 3
# speedup vs baseline: 1.3978x; 1.3978x over previous
"""Multi-head causal self-attention on 8 Trainium2 NeuronCores.

Problem: B=4, T=2048, D=1024, H=16 heads, Hd=64. fp32.
Sharding: core c handles batch b = c//2 and head-group g = c%2 (8 heads,
512 channels). Each core computes a partial output (its head-group's
contribution to x @ Wo); the host sums head-group pairs and adds bo.

Per-core algorithm (all layouts chosen so no on-chip transposes are
needed after the single x -> x^T transpose):
  x^T  [D=1024, T]   via XBAR DMA transpose (weights are DMA'd first so
                     nothing queues behind the 16 transposes)
  Q^T  [C=512, T]    = matmul(lhsT=Wq chunk, rhs=x^T)   (head h at partitions
  K^T  [C=512, T]      64*(h%2) .. of chunk h//2)
  V'   [T, 8*65]     = matmul(lhsT=x^T chunk, rhs=Wv), per head [V(64) | 1]
  S^T  [k,q]         = matmul(lhsT=K^T block, rhs=Q^T span); the two heads
                       of a pair are row-tiled (partitions 0-63 / 64-127)
                       and run concurrently in the PE array
  E = exp(S^T/8)     on ScalarE, PSUM->SBUF; diagonal blocks get a
                     multiplicative staircase mask on their first 128
                     columns only (the rest of the window is always valid)
  ctx' [65, q]       = matmul(lhsT=V' block, rhs=E)  accumulated over k blocks
                       row 64 = softmax denominator (ones-column trick)
  ctx^T normalized via reciprocal + DRAM-bounce partition broadcast; the
                     ctx' PSUM accumulators are copied to SBUF first so the
                     single PSUM bank per head frees immediately
  out  [T, D]        = matmul(lhsT=ctx^T chunk, rhs=Wo chunk), bf16 DMA out

Phase C (attention) is ScalarE-exp-throughput-bound (~1us/k-block), so
the emission order feeds the Tile scheduler coarse filler blocks it can
slot into C's PE idle time:
  - K/Q/V' groups for (hp0, span s) are emitted just before C(hp0, s), so
    the first exp starts ~25us into the kernel;
  - QK groups for hp 1..3 are emitted between the C(hp) phases and get
    scheduled into earlier C spans' PE gaps;
  - hp3 runs its spans DESCENDING with the output projection for span s
    emitted right after norm(3, s), so out-proj overlaps the remaining
    attention work instead of serializing at the end.
PSUM budget: shared V'/QK/out-proj pool 2 banks + S^T 4 + ctx' 2 = 8.
Causality: only k-blocks with k0 <= q_span_end are computed.
"""

import sys

for _p in ("/opt/trn_rl_repo", "/root/.axon_site/_ro/trn_rl_repo"):
    if _p not in sys.path:
        sys.path.append(_p)

import numpy as np

import concourse.bacc as bacc
import concourse.mybir as mybir
import concourse.tile as tile
from concourse.bass_utils import run_bass_kernel_spmd

FP32 = mybir.dt.float32
BF16 = mybir.dt.bfloat16
P = 128
T = 2048  # sequence length
D = 1024  # model dim
C = 512   # channels per core (8 heads)
H = 8     # heads per core
HD = 64   # head dim
N_CORES = 8
NSPAN = 4          # q spans of 512
SPAN = 512
NKB = 16           # k blocks of 128

_program = None


def _build():
    nc = bacc.Bacc()
    x_d = nc.declare_dram_parameter("x", [T, D], BF16, isOutput=False)
    wq_d = nc.declare_dram_parameter("wq", [D, C], BF16, isOutput=False)
    wk_d = nc.declare_dram_parameter("wk", [D, C], BF16, isOutput=False)
    wv_d = nc.declare_dram_parameter("wv", [D, C], BF16, isOutput=False)
    wo_d = nc.declare_dram_parameter("wo", [C, D], BF16, isOutput=False)
    mask_d = nc.declare_dram_parameter("mask", [P, 1024], BF16, isOutput=False)
    out_d = nc.declare_dram_parameter("out", [T, D], BF16, isOutput=True)

    Exp = mybir.ActivationFunctionType.Exp

    from contextlib import ExitStack

    with tile.TileContext(nc) as tc, ExitStack() as persist:
        const_pool = persist.enter_context(tc.tile_pool(name="const", bufs=1))
        qkt_pool = persist.enter_context(tc.tile_pool(name="qkt", bufs=1))
        vp_pool = persist.enter_context(tc.tile_pool(name="vp", bufs=1))
        persist_w = persist.enter_context(tc.tile_pool(name="pw", bufs=1))
        ctxT_pool = persist.enter_context(tc.tile_pool(name="ctxT", bufs=1))
        xt_pool = persist.enter_context(tc.tile_pool(name="xt", bufs=1))

        # ---- persistent SBUF tiles --------------------------------------
        mask_sb = const_pool.tile([P, 1024], BF16, tag="mask")
        wv_sb = persist_w.tile([P, 8, C], BF16, tag="wv")
        wq_sb = persist_w.tile([P, 8, C], BF16, tag="wq")
        wk_sb = persist_w.tile([P, 8, C], BF16, tag="wk")
        wo_sb = persist_w.tile([P, 4, D], BF16, tag="wo")
        qt = [qkt_pool.tile([P, T], BF16, tag=f"qt{i}", name=f"qt{i}") for i in range(4)]
        kt = [qkt_pool.tile([P, T], BF16, tag=f"kt{i}", name=f"kt{i}") for i in range(4)]
        vp = [vp_pool.tile([P, H * 65], BF16, tag=f"vp{t}", name=f"vp{t}") for t in range(NKB)]
        ctxT = [ctxT_pool.tile([P, T], BF16, tag=f"ct{i}", name=f"ct{i}")
                for i in range(4)]
        xt = [xt_pool.tile([P, T], BF16, tag=f"xt{j}", name=f"xt{j}") for j in range(8)]

        # ---- startup DMAs: weights first (small, needed early), then the
        # 16 x-transposes back-to-back on the same ring (DMA transposes
        # serialize against all other in-flight DMAs, so keep them
        # contiguous), wo last (only needed by the output projection).
        nc.sync.dma_start(wv_sb[:], wv_d.rearrange("(o p) c -> p o c", p=P))
        nc.sync.dma_start(wq_sb[:], wq_d.rearrange("(o p) c -> p o c", p=P))
        nc.sync.dma_start(wk_sb[:], wk_d.rearrange("(o p) c -> p o c", p=P))
        nc.sync.dma_start(mask_sb[:], mask_d[:])
        for th in range(2):
            for j in range(8):
                tsl = slice(th * (T // 2), (th + 1) * (T // 2))
                nc.sync.dma_start_transpose(
                    xt[j][:, tsl], x_d[tsl, j * P:(j + 1) * P])
        nc.sync.dma_start(wo_sb[:], wo_d.rearrange("(o p) d -> p o d", p=P))

        # ones columns of V' (value 1.0 at element 64 of each head block)
        for t in range(NKB):
            nc.gpsimd.memset(vp[t][:], 1.0)

        with (
            tc.tile_pool(name="proj", bufs=2, space="PSUM") as proj_pool,
            tc.tile_pool(name="stps", bufs=2, space="PSUM") as st_pool,
            tc.tile_pool(name="csA", bufs=1, space="PSUM") as csA_pool,
            tc.tile_pool(name="csB", bufs=1, space="PSUM") as csB_pool,
            tc.tile_pool(name="epool", bufs=6) as e_pool,
            tc.tile_pool(name="npool", bufs=2) as n_pool,
            tc.tile_pool(name="rdram", bufs=2, space="DRAM") as rdram_pool,
            tc.tile_pool(name="opool", bufs=2) as o_pool,
        ):
            def emit_vprime(t):
                # V' for token block t: [128t, 8*65] with ones col at 64
                ps = proj_pool.tile([P, C], FP32, tag="proj")
                for j in range(8):
                    nc.tensor.matmul(
                        ps[:],
                        xt[j][:, t * P:(t + 1) * P],
                        wv_sb[:, j, :],
                        start=(j == 0), stop=(j == 7),
                    )
                dst = vp[t].rearrange("p (h e) -> p h e", e=65)[:, :, 0:64]
                nc.vector.tensor_copy(dst, ps.rearrange("p (h e) -> p h e", e=64))

            def emit_qk_group(dst, wsb, hp, s):
                ps = proj_pool.tile([P, SPAN], FP32, tag="proj")
                for j in range(8):
                    nc.tensor.matmul(
                        ps[:],
                        wsb[:, j, hp * P:(hp + 1) * P],
                        xt[j][:, s * SPAN:(s + 1) * SPAN],
                        start=(j == 0), stop=(j == 7),
                    )
                nc.vector.tensor_copy(dst[hp][:, s * SPAN:(s + 1) * SPAN], ps[:])

            def emit_attn_span(hp, s):
                hA, hB = 2 * hp, 2 * hp + 1
                csA = csA_pool.tile([P, SPAN], FP32, tag="csA")
                csB = csB_pool.tile([P, SPAN], FP32, tag="csB")
                nkb = 4 * s + 4
                for kb in range(nkb):
                    ksl = slice(kb * P, (kb + 1) * P)
                    d = max(0, kb - 4 * s)      # diagonal offset 0..3
                    q0 = s * SPAN + 128 * d     # valid q start
                    w = SPAN - 128 * d          # valid width
                    qsl = slice(q0, (s + 1) * SPAN)
                    st = st_pool.tile([P, 1024], FP32, tag="st")
                    st3 = st.rearrange("p (b q) -> p b q", b=2)[:, :, 0:w]
                    # the two heads run concurrently (row-tiled at
                    # partitions 0-63 / 64-127)
                    nc.tensor.matmul(st[:, 0:w], kt[hp][0:64, ksl],
                                     qt[hp][0:64, qsl],
                                     start=True, stop=True)
                    nc.tensor.matmul(st[:, 512:512 + w], kt[hp][64:128, ksl],
                                     qt[hp][64:128, qsl],
                                     start=True, stop=True)
                    e = e_pool.tile([P, 1024], BF16, tag="e")
                    e3 = e.rearrange("p (b q) -> p b q", b=2)[:, :, 0:w]
                    nc.scalar.activation(e3, st3, Exp, scale=0.125)
                    if d > 0 or kb == 4 * s:
                        # staircase only affects the first 128 columns of
                        # the valid window (beyond that q-k >= 128 always)
                        e3m = e.rearrange("p (b q) -> p b q", b=2)[:, :, 0:128]
                        m3 = mask_sb[:, None, 384:512]
                        nc.vector.tensor_mul(
                            e3m, e3m, m3.to_broadcast((P, 2, 128)))
                    co = 128 * d
                    nc.tensor.matmul(csA[0:65, co:SPAN],
                                     vp[kb][:, hA * 65:(hA + 1) * 65],
                                     e[:, 0:w],
                                     start=(kb == 0), stop=(kb == nkb - 1))
                    nc.tensor.matmul(csB[0:65, co:SPAN],
                                     vp[kb][:, hB * 65:(hB + 1) * 65],
                                     e[:, 512:512 + w],
                                     start=(kb == 0), stop=(kb == nkb - 1))
                # Copy the accumulators to SBUF immediately so the csA/csB
                # banks free for the next span; normalize from the copy.
                # rows 0..63 / row 64 (ones-column rowsum).
                # reciprocal_approx_fast is broken at nonzero base
                # partition: broadcast first (DRAM bounce), recip at 0.
                qsl = slice(s * SPAN, (s + 1) * SPAN)
                cs = n_pool.tile([P, 1024], FP32, tag="cs")
                rsA = n_pool.tile([P, SPAN], FP32, tag="rsA")
                rsB = n_pool.tile([P, SPAN], FP32, tag="rsB")
                rrA = n_pool.tile([P, SPAN], FP32, tag="rrA")
                rrB = n_pool.tile([P, SPAN], FP32, tag="rrB")
                tmpB = n_pool.tile([P, SPAN], BF16, tag="tmpB")
                nc.vector.tensor_copy(cs[0:65, 0:512], csA[0:65, :])
                nc.vector.tensor_copy(cs[0:65, 512:1024], csB[0:65, :])
                rd = rdram_pool.tile([1024], FP32, tag="rd")
                nc.sync.dma_start(rd[None, :], cs[64:65, :])
                nc.sync.dma_start(
                    rsA[0:64, :], rd[None, 0:512].to_broadcast((64, 512)))
                nc.sync.dma_start(
                    rsB[0:64, :], rd[None, 512:1024].to_broadcast((64, 512)))
                nc.vector.reciprocal_approx_fast(rrA[0:64, :], rsA[0:64, :])
                nc.vector.reciprocal_approx_fast(rrB[0:64, :], rsB[0:64, :])
                nc.vector.tensor_mul(ctxT[hp][0:64, qsl],
                                     cs[0:64, 0:512], rrA[0:64, :])
                nc.vector.tensor_mul(tmpB[0:64, :],
                                     cs[0:64, 512:1024], rrB[0:64, :])
                nc.sync.dma_start(ctxT[hp][64:128, qsl], tmpB[0:64, :])

            def emit_out_span(s):
                # output projection for the 4 token blocks of span s
                for qb in range(4 * s, 4 * s + 4):
                    ot = o_pool.tile([P, 2, SPAN], BF16, tag="ot")
                    for nh in range(2):
                        ps = proj_pool.tile([P, SPAN], FP32, tag="proj")
                        for hp in range(4):
                            nc.tensor.matmul(
                                ps[:],
                                ctxT[hp][:, qb * P:(qb + 1) * P],
                                wo_sb[:, hp, nh * SPAN:(nh + 1) * SPAN],
                                start=(hp == 0), stop=(hp == 3),
                            )
                        nc.vector.tensor_copy(ot[:, nh, :], ps[:])
                    eng = nc.scalar if s == 0 and qb % 2 == 1 else nc.sync
                    eng.dma_start(
                        out_d[qb * P:(qb + 1) * P, :],
                        ot.rearrange("p a b -> p (a b)"))

            # ---- emission schedule ------------------------------------
            # hp0 interleaves its own K/Q/V' groups per span so attention
            # (and with it ScalarE) starts as early as possible.
            for s in range(NSPAN):
                emit_qk_group(kt, wk_sb, 0, s)
                emit_qk_group(qt, wq_sb, 0, s)
                for t in range(4 * s, 4 * s + 4):
                    emit_vprime(t)
                emit_attn_span(0, s)
            for hp in (1, 2):
                for s in range(NSPAN):
                    emit_qk_group(kt, wk_sb, hp, s)
                    emit_qk_group(qt, wq_sb, hp, s)
                for s in range(NSPAN):
                    emit_attn_span(hp, s)
            for s in range(NSPAN):
                emit_qk_group(kt, wk_sb, 3, s)
                emit_qk_group(qt, wq_sb, 3, s)
            for s in reversed(range(NSPAN)):
                emit_attn_span(3, s)
                emit_out_span(s)

    nc.compile()
    return nc


def _get_program():
    global _program
    if _program is None:
        _program = _build()
    return _program


def _make_mask():
    import ml_dtypes
    j = np.arange(1024)[None, :]
    k = np.arange(P)[:, None]
    return np.where(j >= k + 384, 1.0, 0.0).astype(ml_dtypes.bfloat16)


def _make_in_maps(x, Wq, Wk, Wv, Wo):
    import ml_dtypes
    bf16 = ml_dtypes.bfloat16
    mask = _make_mask()
    in_maps = []
    for c in range(N_CORES):
        b, g = c // 2, c % 2
        cols = slice(g * C, (g + 1) * C)
        in_maps.append({
            "x": np.ascontiguousarray(np.asarray(x[b], np.float32).astype(bf16)),
            "wq": np.ascontiguousarray(np.asarray(Wq[:, cols], np.float32).astype(bf16)),
            "wk": np.ascontiguousarray(np.asarray(Wk[:, cols], np.float32).astype(bf16)),
            "wv": np.ascontiguousarray(np.asarray(Wv[:, cols], np.float32).astype(bf16)),
            "wo": np.ascontiguousarray(np.asarray(Wo[cols, :], np.float32).astype(bf16)),
            "mask": mask,
        })
    return in_maps


def _combine(results, bo, B):
    out = np.empty((B, T, D), dtype=np.float32)
    bo = np.asarray(bo, dtype=np.float32)
    for b in range(B):
        out[b] = (results[2 * b]["out"].astype(np.float32)
                  + results[2 * b + 1]["out"].astype(np.float32) + bo)
    return out


def kernel(x, Wq, Wk, Wv, Wo, bo):
    x = np.asarray(x)
    nc = _get_program()
    in_maps = _make_in_maps(x, Wq, Wk, Wv, Wo)
    res = run_bass_kernel_spmd(nc, in_maps, core_ids=list(range(N_CORES)))
    return _combine(res.results, bo, x.shape[0])


def kernel_traced(x, Wq, Wk, Wv, Wo, bo):
    """Like kernel() but also returns the BassKernelResults (with
    exec_time_ns when NTFF tracing is available)."""
    x = np.asarray(x)
    nc = _get_program()
    in_maps = _make_in_maps(x, Wq, Wk, Wv, Wo)
    res = run_bass_kernel_spmd(nc, in_maps, core_ids=list(range(N_CORES)),
                               trace=True)
    return _combine(res.results, bo, x.shape[0]), res
